# revision 19
# baseline (speedup 1.0000x reference)
"""Trainium2 Bass kernel for nn_CFGEmbeder (masked attention pooling).

Reference (per sample, B=128, N=512 nodes, H=512):
    h      = tanh(code_feat @ W_sa + b_sa)          [N, H]
    scores = h @ w_sc (+ b_sc)                      [N]
    attn   = softmax(scores over valid nodes)       [N]
    out    = tanh(attn @ code_feat)                 [H]

Only ~50% of nodes are valid (node_mask); the reference computes the rest
and discards them.  This kernel packs the valid nodes host-side so the
device only touches real work:

  * Samples are sorted by valid count and dealt round-robin to the 8 cores
    (rank r -> core r%8, slot r//8).  Slot j is padded to the max count in
    its rank group (rounded to 16), so slot widths/offsets are identical
    on every core and can be baked into the (SPMD) program as constants.
    The host un-shuffles output rows at the end.
  * Per core the 16 slots are packed into one node axis of K_tot columns
    (~4.2K vs 8K dense), split into halves A (slots 0-7) / B (8-15) that
    are software-pipelined: half A's softmax/pooling tail overlaps half
    B's matmuls.
  * b_sc is dropped (softmax shift invariance).  b_sa==0 takes a fused
    wide-ACT path; nonzero b_sa falls back to per-m-chunk ACTs with bias.

Device pipeline per half (all matmuls fp16, fp32 PSUM):
  mm1   hT[m,i] = sum_k W[k,m].T xT[k,i] per 512-col chunk; tanh fused on
        ScalarE over 2-bank psum pairs -> th fp16.
  score M=1 matvecs (1-col LDWEIGHTS is ~free): row = sum_m wsc[m].T th,
        psum row DMA'd straight to a DRAM scratch.
  smax  scratch reloaded as per-slot rows [8, SMAX] (cross-partition move
        needs a DRAM bounce), masked softmax via (s+1000)*mask trick.
  pool  attn rows -> PE transpose -> attnT columns; out[s] = sum_c
        attnT[:,c,s].T @ xnat[s,c] with 4 samples packed per psum bank via
        col-tiling (tile_position=(0,32j)); final tanh on whole banks,
        DMA of the 4 live rows each.
"""

from contextlib import ExitStack

import numpy as np

import concourse.bass as bass
import concourse.bacc as bacc
import concourse.mybir as mybir
import concourse.tile as tile
from concourse.bass_utils import run_bass_kernel_spmd

F16 = mybir.dt.float16
F32 = mybir.dt.float32

B, N, H = 128, 512, 512
NCORES = 8
S = B // NCORES          # 16 samples (slots) per core
HS = S // 2              # slots per half
KC = H // 128            # 4 contraction chunks
MC = H // 128            # 4 m chunks
CHUNK = 512              # node columns per mm1 chunk
SHIFT = 1000.0           # (scores + SHIFT) * mask; softmax shift invariant

Tanh = mybir.ActivationFunctionType.Tanh
Exp = mybir.ActivationFunctionType.Exp
Alu = mybir.AluOpType


def make_plan(node_mask):
    """Slot assignment + baked widths from the mask (shared across cores)."""
    k = node_mask.astype(bool).sum(1)                     # [B] valid counts
    order = np.argsort(-k, kind="stable")                 # desc by count
    # slot j on core i holds sample order[j*NCORES + i]
    slot_w = []
    for j in range(S):
        grp = k[order[j * NCORES:(j + 1) * NCORES]]
        slot_w.append(max(16, int(np.ceil(grp.max() / 16) * 16)))
    offs = np.concatenate([[0], np.cumsum(slot_w)]).astype(int)
    halves = []
    for h in range(2):
        base = offs[h * HS]
        kh = offs[(h + 1) * HS] - base
        widths = []
        left = kh
        while left > 0:
            widths.append(min(CHUNK, left))
            left -= CHUNK
        halves.append(dict(
            base=base, k=kh, chunks=widths,
            smax=max(slot_w[h * HS:(h + 1) * HS]),
            slot_w=slot_w[h * HS:(h + 1) * HS],
            slot_off=[offs[h * HS + j] - base for j in range(HS)],
        ))
    cj = [int(np.ceil(w / 128)) for w in slot_w]
    cj_off = np.concatenate([[0], np.cumsum(cj)]).astype(int)
    return dict(order=order, slot_w=slot_w, offs=offs, k_tot=int(offs[-1]),
                halves=halves, cj=cj, cj_off=cj_off, cj_tot=int(cj_off[-1]))


def build_program(plan, bsa_zero):
    k_tot = plan["k_tot"]
    cj_tot = plan["cj_tot"]
    cj = plan["cj"]
    cj_off = plan["cj_off"]

    nch = sum(len(hp["chunks"]) for hp in plan["halves"])

    nc = bacc.Bacc(trn_type="TRN2", target_bir_lowering=False,
                   num_devices=NCORES)

    xt_h = nc.dram_tensor("xt", [128, nch, KC, CHUNK], F16,
                          kind="ExternalInput")
    xn_h = nc.dram_tensor("xn", [128, cj_tot, H], F16, kind="ExternalInput")
    w_h = nc.dram_tensor("w_sa", [128, KC, H], F16, kind="ExternalInput")
    bsa_h = nc.dram_tensor("b_sa", [128, MC], F32, kind="ExternalInput")
    wsc_h = nc.dram_tensor("w_sc", [128, MC], F16, kind="ExternalInput")
    msk_h = [nc.dram_tensor(f"mask{h}", [HS, plan["halves"][h]["smax"]], F32,
                            kind="ExternalInput") for h in range(2)]
    id_h = nc.dram_tensor("ident", [16, 16], F32, kind="ExternalInput")
    out_h = nc.dram_tensor("out", [S, H], F32, kind="ExternalOutput")
    sc_h = nc.dram_tensor("score_scratch", [k_tot], F32)

    with tile.TileContext(nc) as tc, ExitStack() as ctx:
        const = ctx.enter_context(tc.tile_pool(name="const", bufs=1))
        xt_p = ctx.enter_context(tc.tile_pool(name="xt", bufs=1))
        xn_p = ctx.enter_context(tc.tile_pool(name="xn", bufs=1))
        th_p = ctx.enter_context(tc.tile_pool(name="th", bufs=4))
        sm_p = ctx.enter_context(tc.tile_pool(name="sm", bufs=1))
        row_p = ctx.enter_context(tc.tile_pool(name="row", bufs=2))
        ph_p = ctx.enter_context(tc.tile_pool(name="ph", bufs=2, space="PSUM"))
        pr_p = ctx.enter_context(tc.tile_pool(name="pr", bufs=1, space="PSUM"))
        pa_p = ctx.enter_context(tc.tile_pool(name="pa", bufs=1, space="PSUM"))
        pt_p = ctx.enter_context(tc.tile_pool(name="pt", bufs=2, space="PSUM"))

        # --- constants (scalar HWDGE: ScalarE is idle at program start) ---
        Wf = const.tile([128, KC, H], F16, name="Wf")
        nc.scalar.dma_start(Wf, w_h.ap())
        wsc = const.tile([128, MC], F16, name="wsc")
        nc.scalar.dma_start(wsc, wsc_h.ap())
        bsa = const.tile([128, MC], F32, name="bsa")
        nc.scalar.dma_start(bsa, bsa_h.ap())
        idf = const.tile([16, 16], F32, name="idf")
        nc.scalar.dma_start(idf, id_h.ap())
        masks = []
        for h in range(2):
            mk = const.tile([HS, plan["halves"][h]["smax"]], F32,
                            name=f"mask{h}")
            nc.scalar.dma_start(mk, msk_h[h].ap())
            masks.append(mk)

        # softmax input tiles: memset early so stale-SBUF NaNs can't leak
        # through the (s+SHIFT)*mask product in the reload slack columns
        sm_tiles = []
        for h in range(2):
            smt = const.tile([HS, plan["halves"][h]["smax"]], F32,
                             name=f"smin{h}")
            nc.vector.memset(smt, 0.0)
            sm_tiles.append(smt)

        # --- x loads ---
        # xt feeds mm1 immediately: chunk-major layout (one contiguous run
        # per partition -> cheap descriptors), chunks rotated across all
        # three DMA queues in consumption order so arrival outpaces mm1
        xt = xt_p.tile([128, nch, KC, CHUNK], F16, name="xt")
        queues = [nc.sync, nc.gpsimd, nc.scalar]
        for c in range(nch):
            q = queues[c % 3]
            q.dma_start(xt[:, c, :, :], xt_h.ap()[:, c, :, :])
        # xnat (pooling, needed much later): halves on the two HWDGE rings
        xn = xn_p.tile([128, cj_tot, H], F16, name="xn")
        cjA = int(cj_off[HS])
        nc.sync.dma_start(xn[:, 0:cjA, :], xn_h.ap()[:, 0:cjA, :])
        nc.scalar.dma_start(xn[:, cjA:, :], xn_h.ap()[:, cjA:, :])

        def half_chunks(h):
            """mm1 + fused tanh + score rows for one half."""
            hp = plan["halves"][h]
            c0 = hp["base"]
            cbase = len(plan["halves"][0]["chunks"]) if h else 0
            for ci, w in enumerate(hp["chunks"]):
                ths = []
                for pair in range(2):
                    ph = ph_p.tile([128, 2, CHUNK], F32, name="ph")
                    for mi in range(2):
                        m = 2 * pair + mi
                        for k in range(KC):
                            nc.tensor.matmul(
                                ph[:, mi, 0:w],
                                lhsT=Wf[:, k, m * 128:(m + 1) * 128],
                                rhs=xt[:, cbase + ci, k, 0:w],
                                start=(k == 0), stop=(k == KC - 1),
                            )
                    th = th_p.tile([128, 2, CHUNK], F16, name="th")
                    if bsa_zero:
                        nc.scalar.activation(th[:, :, 0:w], ph[:, :, 0:w],
                                             Tanh)
                    else:
                        for mi in range(2):
                            m = 2 * pair + mi
                            nc.scalar.activation(th[:, mi, 0:w],
                                                 ph[:, mi, 0:w], Tanh,
                                                 bias=bsa[:, m:m + 1])
                    ths.append(th)
                pr = pr_p.tile([1, CHUNK], F32, name="pr")
                for m in range(MC):
                    nc.tensor.matmul(
                        pr[0:1, 0:w],
                        lhsT=wsc[:, m:m + 1],
                        rhs=ths[m // 2][:, m % 2, 0:w],
                        start=(m == 0), stop=(m == MC - 1),
                    )
                srow = row_p.tile([1, CHUNK], F32, name="srow")
                nc.vector.tensor_copy(srow[:, 0:w], pr[0:1, 0:w])
                nc.gpsimd.dma_start(sc_h.ap()[c0:c0 + w], srow[:, 0:w])
                c0 += w

        def half_tail(h):
            """softmax + attn transpose + pooling + output for one half."""
            hp = plan["halves"][h]
            smax = hp["smax"]
            ncc = (smax + 127) // 128
            smt = sm_tiles[h]
            for j in range(HS):
                o, wj = hp["slot_off"][j], hp["slot_w"][j]
                nc.gpsimd.dma_start(smt[j:j + 1, 0:wj],
                                    sc_h.ap()[hp["base"] + o:
                                              hp["base"] + o + wj])
            msk = masks[h]
            m1 = sm_p.tile([HS, smax], F32, name=f"m1_{h}")
            nc.vector.scalar_tensor_tensor(m1, smt, SHIFT, msk,
                                           op0=Alu.add, op1=Alu.mult)
            nmax = sm_p.tile([HS, 1], F32, name=f"nmax{h}")
            nc.vector.tensor_reduce(nmax, m1, axis=mybir.AxisListType.X,
                                    op=Alu.max, negate=True)
            ex = sm_p.tile([HS, smax], F32, name=f"ex{h}")
            esum = sm_p.tile([HS, 1], F32, name=f"esum{h}")
            nc.scalar.activation(ex, m1, Exp, bias=nmax, accum_out=esum)
            rinv = sm_p.tile([HS, 1], F32, name=f"rinv{h}")
            nc.vector.reciprocal(rinv, esum)
            attn = sm_p.tile([HS, smax], F32, name=f"attn{h}")
            nc.vector.tensor_scalar_mul(attn, ex, rinv)

            paT = pa_p.tile([128, ncc, HS], F32, name="paT")
            for c in range(ncc):
                wcol = min(128, smax - c * 128)
                nc.tensor.transpose(paT[0:wcol, c, :],
                                    attn[:, c * 128:c * 128 + wcol],
                                    idf[0:HS, 0:HS])
            attnT = sm_p.tile([128, ncc, HS], F16, name=f"attnT{h}")
            nc.vector.tensor_copy(attnT, paT)

            for g in range(HS // 4):
                pp = pt_p.tile([128, H], F32, name="pp")
                cmax = max(cj[h * HS + g * 4 + jj] for jj in range(4))
                for c in range(cmax):
                    for jj in range(4):
                        j = g * 4 + jj
                        sj = h * HS + j
                        if c >= cj[sj]:
                            continue
                        nc.tensor.matmul(
                            pp[32 * jj:32 * jj + 1, :],
                            lhsT=attnT[:, c, j:j + 1],
                            rhs=xn[:, cj_off[sj] + c, :],
                            start=(c == 0), stop=(c == cj[sj] - 1),
                            tile_position=(0, 32 * jj),
                        )
                orow = row_p.tile([128, H], F32, name="orow")
                nc.scalar.activation(orow, pp, Tanh)
                nc.gpsimd.dma_start(
                    out_h.ap().rearrange("(g four) h -> four g h", four=4)
                    [:, h * 2 + g, :],
                    orow[0:97:32, :])

        # software pipeline: A's tail overlaps B's chunk work
        half_chunks(0)
        half_chunks(1)
        half_tail(0)
        half_tail(1)

    nc.finalize()
    return nc


_CACHE = {}


def _get_nc(plan, bsa_zero):
    key = (tuple(plan["slot_w"]), bsa_zero)
    if key not in _CACHE:
        _CACHE[key] = build_program(plan, bsa_zero)
    return _CACHE[key]


def make_in_maps(plan, code_feat, node_mask, W_sa, b_sa, w_sc):
    x16 = np.asarray(code_feat, dtype=np.float16)
    k = np.asarray(node_mask).astype(bool)
    order = plan["order"]
    slot_w = plan["slot_w"]
    k_tot = plan["k_tot"]
    cj, cj_off, cj_tot = plan["cj"], plan["cj_off"], plan["cj_tot"]

    w16 = np.asarray(W_sa, dtype=np.float16)
    wf = np.ascontiguousarray(
        w16.reshape(KC, 128, H).transpose(1, 0, 2))          # [p,k,m]
    wsc16 = np.asarray(w_sc, dtype=np.float16).reshape(MC, 128).T.copy()
    bsa32 = np.asarray(b_sa, dtype=np.float32).reshape(MC, 128).T.copy()
    ident = np.eye(16, dtype=np.float32)

    in_maps = []
    for i in range(NCORES):
        xpk = np.zeros((k_tot, H), dtype=np.float16)
        xnc = np.zeros((128, cj_tot, H), dtype=np.float16)
        masks = [np.zeros((HS, plan["halves"][h]["smax"]), dtype=np.float32)
                 for h in range(2)]
        for j in range(S):
            s = order[j * NCORES + i]
            idx = np.nonzero(k[s])[0]
            o = plan["offs"][j]
            xpk[o:o + len(idx)] = x16[s, idx]
            h, jj = divmod(j, HS)
            masks[h][jj, 0:len(idx)] = 1.0
            pad = np.zeros((cj[j] * 128, H), dtype=np.float16)
            pad[0:len(idx)] = x16[s, idx]
            xnc[:, cj_off[j]:cj_off[j] + cj[j], :] = (
                pad.reshape(cj[j], 128, H).transpose(1, 0, 2))
        # chunk-major xt: [p, chunk, k, i], chunks re-based per half
        parts = []
        for h in range(2):
            hp = plan["halves"][h]
            nchh = len(hp["chunks"])
            xph = np.zeros((nchh * CHUNK, H), dtype=np.float16)
            xph[0:hp["k"]] = xpk[hp["base"]:hp["base"] + hp["k"]]
            parts.append(xph.reshape(nchh, CHUNK, KC, 128)
                         .transpose(3, 0, 2, 1))
        xt = np.ascontiguousarray(np.concatenate(parts, axis=1))
        in_maps.append({
            "xt": xt, "xn": xnc, "w_sa": wf, "b_sa": bsa32, "w_sc": wsc16,
            "mask0": masks[0], "mask1": masks[1], "ident": ident,
        })
    return in_maps


def kernel(code_feat, node_mask, W_sa, b_sa, w_sc, b_sc=None, **_ignored):
    code_feat = np.asarray(code_feat)
    node_mask = np.asarray(node_mask)
    plan = make_plan(node_mask)
    bsa_zero = not np.any(np.asarray(b_sa))
    nc = _get_nc(plan, bsa_zero)
    in_maps = make_in_maps(plan, code_feat, node_mask, W_sa, b_sa, w_sc)
    res = run_bass_kernel_spmd(nc, in_maps, list(range(NCORES)))
    out = np.empty((B, H), dtype=np.float32)
    order = plan["order"]
    for i in range(NCORES):
        for j in range(S):
            out[order[j * NCORES + i]] = res.results[i]["out"][j]
    return out


# revision 23
# speedup vs baseline: 1.1028x; 1.1028x over previous
"""Trainium2 Bass kernel for nn_CFGEmbeder (masked attention pooling).

Reference (per sample, B=128, N=512 nodes, H=512):
    h      = tanh(code_feat @ W_sa + b_sa)          [N, H]
    scores = h @ w_sc (+ b_sc)                      [N]
    attn   = softmax(scores over valid nodes)       [N]
    out    = tanh(attn @ code_feat)                 [H]

Only ~50% of nodes are valid (node_mask); the reference computes the rest
and discards them.  This kernel packs the valid nodes host-side so the
device only touches real work:

  * Samples are sorted by valid count and dealt round-robin to the 8 cores
    (rank r -> core r%8, slot r//8).  Slot j is padded to the max count in
    its rank group (rounded to 16), so slot widths/offsets are identical
    on every core and can be baked into the (SPMD) program as constants.
    The host un-shuffles output rows at the end.
  * Per core the 16 slots are packed into one node axis of K_tot columns
    (~4.2K vs 8K dense), split into halves A (slots 0-7) / B (8-15) that
    are software-pipelined: half A's softmax/pooling tail overlaps half
    B's matmuls.
  * b_sc is dropped (softmax shift invariance).  b_sa==0 takes a fused
    wide-ACT path; nonzero b_sa falls back to per-m-chunk ACTs with bias.

Device pipeline per half (all matmuls fp16, fp32 PSUM):
  mm1   hT[m,i] = sum_k W[k,m].T xT[k,i] per 512-col chunk; tanh fused on
        ScalarE over 2-bank psum pairs -> th fp16.
  score M=1 matvecs (1-col LDWEIGHTS is ~free): row = sum_m wsc[m].T th,
        psum row DMA'd straight to a DRAM scratch.
  smax  scratch reloaded as per-slot rows [8, SMAX] (cross-partition move
        needs a DRAM bounce), masked softmax via (s+1000)*mask trick.
  pool  attn rows -> PE transpose -> attnT columns; out[s] = sum_c
        attnT[:,c,s].T @ xnat[s,c] with 4 samples packed per psum bank via
        col-tiling (tile_position=(0,32j)); final tanh on whole banks,
        DMA of the 4 live rows each.
"""

from contextlib import ExitStack

import numpy as np

import concourse.bass as bass
import concourse.bacc as bacc
import concourse.mybir as mybir
import concourse.tile as tile
from concourse.bass_utils import run_bass_kernel_spmd

F16 = mybir.dt.float16
F32 = mybir.dt.float32

B, N, H = 128, 512, 512
NCORES = 8
S = B // NCORES          # 16 samples (slots) per core
HS = S // 2              # slots per half
KC = H // 128            # 4 contraction chunks
MC = H // 128            # 4 m chunks
CHUNK = 512              # node columns per mm1 chunk
SHIFT = 1000.0           # (scores + SHIFT) * mask; softmax shift invariant

Tanh = mybir.ActivationFunctionType.Tanh
Exp = mybir.ActivationFunctionType.Exp
Alu = mybir.AluOpType


def make_plan(node_mask):
    """Slot assignment + baked widths from the mask (shared across cores)."""
    k = node_mask.astype(bool).sum(1)                     # [B] valid counts
    order = np.argsort(-k, kind="stable")                 # desc by count
    # slot j on core i holds sample order[j*NCORES + i]
    slot_w = []
    for j in range(S):
        grp = k[order[j * NCORES:(j + 1) * NCORES]]
        slot_w.append(max(16, int(np.ceil(grp.max() / 16) * 16)))
    offs = np.concatenate([[0], np.cumsum(slot_w)]).astype(int)
    halves = []
    for h in range(2):
        base = offs[h * HS]
        kh = offs[(h + 1) * HS] - base
        widths = []
        left = kh
        while left > 0:
            widths.append(min(CHUNK, left))
            left -= CHUNK
        halves.append(dict(
            base=base, k=kh, chunks=widths,
            smax=max(slot_w[h * HS:(h + 1) * HS]),
            slot_w=slot_w[h * HS:(h + 1) * HS],
            slot_off=[offs[h * HS + j] - base for j in range(HS)],
        ))
    cj = [int(np.ceil(w / 128)) for w in slot_w]
    cj_off = np.concatenate([[0], np.cumsum(cj)]).astype(int)
    return dict(order=order, slot_w=slot_w, offs=offs, k_tot=int(offs[-1]),
                halves=halves, cj=cj, cj_off=cj_off, cj_tot=int(cj_off[-1]))


def build_program(plan, bsa_zero):
    k_tot = plan["k_tot"]
    cj_tot = plan["cj_tot"]
    cj = plan["cj"]
    cj_off = plan["cj_off"]

    nch = sum(len(hp["chunks"]) for hp in plan["halves"])

    nc = bacc.Bacc(trn_type="TRN2", target_bir_lowering=False,
                   num_devices=NCORES)

    xt_h = nc.dram_tensor("xt", [128, nch, KC, CHUNK], F16,
                          kind="ExternalInput")
    xn_h = nc.dram_tensor("xn", [128, cj_tot, H], F16, kind="ExternalInput")
    w_h = nc.dram_tensor("w_sa", [128, KC, H], F16, kind="ExternalInput")
    bsa_h = nc.dram_tensor("b_sa", [128, MC], F32, kind="ExternalInput")
    wsc_h = nc.dram_tensor("w_sc", [128, MC], F16, kind="ExternalInput")
    msk_h = [nc.dram_tensor(f"mask{h}", [HS, plan["halves"][h]["smax"]], F32,
                            kind="ExternalInput") for h in range(2)]
    id_h = nc.dram_tensor("ident", [16, 16], F32, kind="ExternalInput")
    out_h = nc.dram_tensor("out", [S, H], F32, kind="ExternalOutput")
    sc_h = nc.dram_tensor("score_scratch", [k_tot], F32)

    with tile.TileContext(nc) as tc, ExitStack() as ctx:
        const = ctx.enter_context(tc.tile_pool(name="const", bufs=1))
        xt_p = ctx.enter_context(tc.tile_pool(name="xt", bufs=1))
        xn_p = ctx.enter_context(tc.tile_pool(name="xn", bufs=1))
        th_p = ctx.enter_context(tc.tile_pool(name="th", bufs=4))
        sm_p = ctx.enter_context(tc.tile_pool(name="sm", bufs=1))
        row_p = ctx.enter_context(tc.tile_pool(name="row", bufs=2))
        ph_p = ctx.enter_context(tc.tile_pool(name="ph", bufs=2, space="PSUM"))
        pr_p = ctx.enter_context(tc.tile_pool(name="pr", bufs=1, space="PSUM"))
        pa_p = ctx.enter_context(tc.tile_pool(name="pa", bufs=1, space="PSUM"))
        pt_p = ctx.enter_context(tc.tile_pool(name="pt", bufs=2, space="PSUM"))

        # --- constants (scalar HWDGE: ScalarE is idle at program start) ---
        Wf = const.tile([128, KC, H], F16, name="Wf")
        nc.scalar.dma_start(Wf, w_h.ap())
        wsc = const.tile([128, MC], F16, name="wsc")
        nc.scalar.dma_start(wsc, wsc_h.ap())
        bsa = const.tile([128, MC], F32, name="bsa")
        nc.scalar.dma_start(bsa, bsa_h.ap())
        idf = const.tile([16, 16], F32, name="idf")
        nc.scalar.dma_start(idf, id_h.ap())
        masks = []
        for h in range(2):
            mk = const.tile([HS, plan["halves"][h]["smax"]], F32,
                            name=f"mask{h}")
            nc.scalar.dma_start(mk, msk_h[h].ap())
            masks.append(mk)

        # softmax input tiles: memset early so stale-SBUF NaNs can't leak
        # through the (s+SHIFT)*mask product in the reload slack columns
        sm_tiles = []
        for h in range(2):
            smt = const.tile([HS, plan["halves"][h]["smax"]], F32,
                             name=f"smin{h}")
            nc.vector.memset(smt, 0.0)
            sm_tiles.append(smt)

        # --- x loads ---
        # xt feeds mm1 immediately: chunk-major layout (one contiguous run
        # per partition -> cheap descriptors), chunks rotated across all
        # three DMA queues in consumption order so arrival outpaces mm1
        xt = xt_p.tile([128, nch, KC, CHUNK], F16, name="xt")
        queues = [nc.sync, nc.gpsimd, nc.scalar]
        for c in range(nch):
            q = queues[c % 3]
            q.dma_start(xt[:, c, :, :], xt_h.ap()[:, c, :, :])
        # xnat (pooling, needed much later): scalar ring after the consts.
        # sync/gpsimd stay shallow -- they carry the latency-critical score
        # bounce+reload chains for halves A and B respectively (same-queue
        # ordering makes the DRAM round-trip safe).
        xn = xn_p.tile([128, cj_tot, H], F16, name="xn")
        cjA = int(cj_off[HS])
        nc.scalar.dma_start(xn[:, 0:cjA, :], xn_h.ap()[:, 0:cjA, :])
        nc.scalar.dma_start(xn[:, cjA:, :], xn_h.ap()[:, cjA:, :])
        half_q = [nc.sync, nc.gpsimd]

        def half_chunks(h):
            """mm1 + fused tanh + score rows for one half."""
            hp = plan["halves"][h]
            c0 = hp["base"]
            cbase = len(plan["halves"][0]["chunks"]) if h else 0
            for ci, w in enumerate(hp["chunks"]):
                ths = []
                for pair in range(2):
                    ph = ph_p.tile([128, 2, CHUNK], F32, name="ph")
                    for mi in range(2):
                        m = 2 * pair + mi
                        for k in range(KC):
                            nc.tensor.matmul(
                                ph[:, mi, 0:w],
                                lhsT=Wf[:, k, m * 128:(m + 1) * 128],
                                rhs=xt[:, cbase + ci, k, 0:w],
                                start=(k == 0), stop=(k == KC - 1),
                            )
                    th = th_p.tile([128, 2, CHUNK], F16, name="th")
                    if bsa_zero:
                        nc.scalar.activation(th[:, :, 0:w], ph[:, :, 0:w],
                                             Tanh)
                    else:
                        for mi in range(2):
                            m = 2 * pair + mi
                            nc.scalar.activation(th[:, mi, 0:w],
                                                 ph[:, mi, 0:w], Tanh,
                                                 bias=bsa[:, m:m + 1])
                    ths.append(th)
                pr = pr_p.tile([1, CHUNK], F32, name="pr")
                for m in range(MC):
                    nc.tensor.matmul(
                        pr[0:1, 0:w],
                        lhsT=wsc[:, m:m + 1],
                        rhs=ths[m // 2][:, m % 2, 0:w],
                        start=(m == 0), stop=(m == MC - 1),
                    )
                srow = row_p.tile([1, CHUNK], F32, name="srow")
                nc.vector.tensor_copy(srow[:, 0:w], pr[0:1, 0:w])
                half_q[h].dma_start(sc_h.ap()[c0:c0 + w], srow[:, 0:w])
                c0 += w

        def tail_soft(h):
            """score reload + masked softmax + attn (no PE work)."""
            hp = plan["halves"][h]
            smax = hp["smax"]
            smt = sm_tiles[h]
            for j in range(HS):
                o, wj = hp["slot_off"][j], hp["slot_w"][j]
                half_q[h].dma_start(smt[j:j + 1, 0:wj],
                                    sc_h.ap()[hp["base"] + o:
                                              hp["base"] + o + wj])
            msk = masks[h]
            m1 = sm_p.tile([HS, smax], F32, name=f"m1_{h}")
            nc.vector.scalar_tensor_tensor(m1, smt, SHIFT, msk,
                                           op0=Alu.add, op1=Alu.mult)
            nmax = sm_p.tile([HS, 1], F32, name=f"nmax{h}")
            nc.vector.tensor_reduce(nmax, m1, axis=mybir.AxisListType.X,
                                    op=Alu.max, negate=True)
            ex = sm_p.tile([HS, smax], F32, name=f"ex{h}")
            esum = sm_p.tile([HS, 1], F32, name=f"esum{h}")
            nc.scalar.activation(ex, m1, Exp, bias=nmax, accum_out=esum)
            rinv = sm_p.tile([HS, 1], F32, name=f"rinv{h}")
            nc.vector.reciprocal(rinv, esum)
            attn = sm_p.tile([HS, smax], F32, name=f"attn{h}")
            nc.vector.tensor_scalar_mul(attn, ex, rinv)
            return attn

        def tail_pool(h, attn):
            """attn transpose + col-tiled pooling + output (PE phase)."""
            hp = plan["halves"][h]
            smax = hp["smax"]
            ncc = (smax + 127) // 128
            paT = pa_p.tile([128, ncc, HS], F32, name="paT")
            for c in range(ncc):
                wcol = min(128, smax - c * 128)
                nc.tensor.transpose(paT[0:wcol, c, :],
                                    attn[:, c * 128:c * 128 + wcol],
                                    idf[0:HS, 0:HS])
            attnT = sm_p.tile([128, ncc, HS], F16, name=f"attnT{h}")
            nc.vector.tensor_copy(attnT, paT)

            for g in range(HS // 4):
                pp = pt_p.tile([128, H], F32, name="pp")
                cmax = max(cj[h * HS + g * 4 + jj] for jj in range(4))
                for c in range(cmax):
                    for jj in range(4):
                        j = g * 4 + jj
                        sj = h * HS + j
                        if c >= cj[sj]:
                            continue
                        nc.tensor.matmul(
                            pp[32 * jj:32 * jj + 1, :],
                            lhsT=attnT[:, c, j:j + 1],
                            rhs=xn[:, cj_off[sj] + c, :],
                            start=(c == 0), stop=(c == cj[sj] - 1),
                            tile_position=(0, 32 * jj),
                        )
                orow = row_p.tile([128, H], F32, name="orow")
                nc.scalar.activation(orow, pp, Tanh)
                half_q[h].dma_start(
                    out_h.ap().rearrange("(g four) h -> four g h", four=4)
                    [:, h * 2 + g, :],
                    orow[0:97:32, :])

        # software pipeline: half A's softmax runs right after its chunks
        # (emitted before B's tanh ACTs so ScalarE's FIFO doesn't starve
        # it), and A's PE tail slots in between B's chunk matmuls
        half_chunks(0)
        attn0 = tail_soft(0)
        half_chunks(1)
        tail_pool(0, attn0)
        attn1 = tail_soft(1)
        tail_pool(1, attn1)

    nc.finalize()
    return nc


_CACHE = {}


def _get_nc(plan, bsa_zero):
    key = (tuple(plan["slot_w"]), bsa_zero)
    if key not in _CACHE:
        _CACHE[key] = build_program(plan, bsa_zero)
    return _CACHE[key]


def make_in_maps(plan, code_feat, node_mask, W_sa, b_sa, w_sc):
    x16 = np.asarray(code_feat, dtype=np.float16)
    k = np.asarray(node_mask).astype(bool)
    order = plan["order"]
    slot_w = plan["slot_w"]
    k_tot = plan["k_tot"]
    cj, cj_off, cj_tot = plan["cj"], plan["cj_off"], plan["cj_tot"]

    w16 = np.asarray(W_sa, dtype=np.float16)
    wf = np.ascontiguousarray(
        w16.reshape(KC, 128, H).transpose(1, 0, 2))          # [p,k,m]
    wsc16 = np.asarray(w_sc, dtype=np.float16).reshape(MC, 128).T.copy()
    bsa32 = np.asarray(b_sa, dtype=np.float32).reshape(MC, 128).T.copy()
    ident = np.eye(16, dtype=np.float32)

    in_maps = []
    for i in range(NCORES):
        xpk = np.zeros((k_tot, H), dtype=np.float16)
        xnc = np.zeros((128, cj_tot, H), dtype=np.float16)
        masks = [np.zeros((HS, plan["halves"][h]["smax"]), dtype=np.float32)
                 for h in range(2)]
        for j in range(S):
            s = order[j * NCORES + i]
            idx = np.nonzero(k[s])[0]
            o = plan["offs"][j]
            xpk[o:o + len(idx)] = x16[s, idx]
            h, jj = divmod(j, HS)
            masks[h][jj, 0:len(idx)] = 1.0
            pad = np.zeros((cj[j] * 128, H), dtype=np.float16)
            pad[0:len(idx)] = x16[s, idx]
            xnc[:, cj_off[j]:cj_off[j] + cj[j], :] = (
                pad.reshape(cj[j], 128, H).transpose(1, 0, 2))
        # chunk-major xt: [p, chunk, k, i], chunks re-based per half
        parts = []
        for h in range(2):
            hp = plan["halves"][h]
            nchh = len(hp["chunks"])
            xph = np.zeros((nchh * CHUNK, H), dtype=np.float16)
            xph[0:hp["k"]] = xpk[hp["base"]:hp["base"] + hp["k"]]
            parts.append(xph.reshape(nchh, CHUNK, KC, 128)
                         .transpose(3, 0, 2, 1))
        xt = np.ascontiguousarray(np.concatenate(parts, axis=1))
        in_maps.append({
            "xt": xt, "xn": xnc, "w_sa": wf, "b_sa": bsa32, "w_sc": wsc16,
            "mask0": masks[0], "mask1": masks[1], "ident": ident,
        })
    return in_maps


def kernel(code_feat, node_mask, W_sa, b_sa, w_sc, b_sc=None, **_ignored):
    code_feat = np.asarray(code_feat)
    node_mask = np.asarray(node_mask)
    plan = make_plan(node_mask)
    bsa_zero = not np.any(np.asarray(b_sa))
    nc = _get_nc(plan, bsa_zero)
    in_maps = make_in_maps(plan, code_feat, node_mask, W_sa, b_sa, w_sc)
    res = run_bass_kernel_spmd(nc, in_maps, list(range(NCORES)))
    out = np.empty((B, H), dtype=np.float32)
    order = plan["order"]
    for i in range(NCORES):
        for j in range(S):
            out[order[j * NCORES + i]] = res.results[i]["out"][j]
    return out
